# revision 1
# baseline (speedup 1.0000x reference)
"""MoE routing kernel for TRN2, 8 NeuronCores (expert-parallel).

Math: out[t] = sum_{e in top2(logits[t])} x[t] @ w_up[e] @ w_down[e]
(reference applies no activation between projections and no prob weighting,
so each expert collapses to one fused matrix W_e = w_up[e] @ w_down[e]).

Distribution:
  phase 1 (expert parallel): core c computes W_c = w_up[c] @ w_down[c]
           in bf16 (fp32 PSUM accumulation), 4.3 G MACs/core.  w_up is
           passed pre-transposed (host layout prep) so both operands load
           with the contraction dim on partitions.
  AllGather of the fused W matrices (bf16), one per d2-half: the first
           fires as soon as the d2-low half of W is accumulated (paced by
           the weight DMA itself), overlapping the second half's matmuls.
  phase 2 (token parallel): core c owns tokens [256c, 256(c+1)); computes
           fp32 routing logits, top-2 mask, and accumulates the 8 masked
           matmuls x_loc @ W_e into PSUM. Output is an exact token shard.
"""

import numpy as np

E = 8
TOPK = 2
D = 1024
I = 4096
T = 2048  # 4*512 tokens
N_CORES = 8
TL = T // N_CORES  # tokens per core (256)
P = 128
BIG = 1.0e30

_cached = {}


def _build():
    import concourse.bass as bass  # noqa: F401
    import concourse.tile as tile
    from concourse import bacc, mybir
    from concourse.masks import make_identity

    f32 = mybir.dt.float32
    bf16 = mybir.dt.bfloat16

    nc = bacc.Bacc("TRN2", target_bir_lowering=False, debug=False, num_devices=N_CORES)
    x_ext = nc.declare_dram_parameter("hidden_states", [TL, D], f32, isOutput=False)
    gate_ext = nc.declare_dram_parameter("gate_w", [E, D], f32, isOutput=False)
    upT_ext = nc.declare_dram_parameter("w_upT", [I, D], f32, isOutput=False)
    down_ext = nc.declare_dram_parameter("w_down", [I, D], f32, isOutput=False)
    out_ext = nc.declare_dram_parameter("out", [TL, D], f32, isOutput=True)

    KB = D // P      # 8  d1 blocks
    KI = I // P      # 32 i blocks
    TB = TL // P     # 2  token blocks
    NCH = D // 512   # 2  d2 chunks

    with tile.TileContext(nc) as tc:
        import contextlib

        with contextlib.ExitStack() as ctx:
            # ---- long-lived small pools ----
            const = ctx.enter_context(tc.tile_pool(name="const", bufs=1))
            outer = ctx.enter_context(tc.tile_pool(name="outer", bufs=1))
            dram = ctx.enter_context(tc.tile_pool(name="dram", bufs=1, space="DRAM"))

            ident = const.tile([P, P], f32)
            make_identity(nc, ident[:])
            ones1 = const.tile([1, P], f32)
            nc.vector.memset(ones1[:], 1.0)
            c127 = const.tile([P, 8], f32)
            nc.vector.memset(c127[:], 127.0)

            xT = outer.tile([P, KB, TL], f32)        # x_loc^T  [d1, t]
            mbc = outer.tile([P, E, TL], f32)        # per-expert masks bcast over partitions
            maskT = outer.tile([1, E, TL], f32)      # mask^T rows, all on partition 0
            mtmp = outer.tile([8, TB, P], f32)
            logits = outer.tile([P, TB, E], f32)
            m1 = outer.tile([P, TB], f32)
            eqbig = outer.tile([P, TB, E], f32)
            l2 = outer.tile([P, TB, E], f32)
            m2 = outer.tile([P, TB], f32)
            mask = outer.tile([P, TB, E], f32)
            gate_wT = outer.tile([P, KB, E], f32)

            # AG bounce buffers, one per d2 half; flattened so every DMA run
            # is >=4KB contiguous per partition.
            ag_in = []
            ag_out = []
            for h in range(NCH):
                gi = dram.tile([P, KB * 512], bf16, name=f"ag_in_{h}")
                go = dram.tile(
                    [E, P, KB * 512], bf16, addr_space="Shared", name=f"ag_out_{h}"
                )
                ag_in.append(gi)
                ag_out.append(go)

            # ---- big weight pools (phase 1) ----
            big_cm = tc.tile_pool(name="big", bufs=1)
            big = big_cm.__enter__()
            upT = big.tile([P, KI, D], bf16)    # up^T  [i, d1]
            down0 = big.tile([P, KI, 512], bf16)  # down[:, :512]
            down1 = big.tile([P, KI, 512], bf16)  # down[:, 512:]
            downs = [down0, down1]

            ph1_cm = tc.tile_pool(name="ph1", bufs=3)
            ph1 = ph1_cm.__enter__()
            psA_cm = tc.tile_pool(name="psA", bufs=4, space="PSUM")
            psA = psA_cm.__enter__()

            # small loads first on the sync queue
            xnat = ph1.tile([P, TB, D], f32, tag="xnat", bufs=1)
            nc.sync.dma_start(xnat[:], x_ext.rearrange("(b p) d -> p b d", p=P))
            gsb = ph1.tile([8, D], f32, tag="gate", bufs=1)
            nc.sync.dma_start(gsb[:], gate_ext[:])

            # weight cast-loads (f32 -> bf16 via SWDGE), quarter-interleaved.
            # down's d2-low half loads first so W[:, :512] (and its AllGather)
            # completes when only ~3/4 of the weight bytes have landed.
            for q in range(8):
                nc.gpsimd.dma_start(
                    upT[:, 4 * q : 4 * (q + 1), :],
                    upT_ext[512 * q : 512 * (q + 1), :].rearrange(
                        "(ko p) n -> p ko n", p=P
                    ),
                )
                nc.gpsimd.dma_start(
                    down0[:, 4 * q : 4 * (q + 1), :],
                    down_ext[512 * q : 512 * (q + 1), 0:512].rearrange(
                        "(ko p) n -> p ko n", p=P
                    ),
                )
            for q in range(8):
                nc.gpsimd.dma_start(
                    down1[:, 4 * q : 4 * (q + 1), :],
                    down_ext[512 * q : 512 * (q + 1), 512:1024].rearrange(
                        "(ko p) n -> p ko n", p=P
                    ),
                )

            # gate_w transpose: [8, 1024] -> [1024, 8]
            for kb in range(KB):
                pt = psA.tile([P, 8], f32, tag="tp")
                nc.tensor.transpose(pt[:], gsb[:, P * kb : P * (kb + 1)], ident[:8, :8])
                nc.vector.tensor_copy(out=gate_wT[:, kb, :], in_=pt[:])

            # x transpose: [256, 1024] -> [1024, 256]
            for tb in range(TB):
                for kb in range(KB):
                    pt = psA.tile([P, P], f32, tag="tp")
                    nc.tensor.transpose(
                        pt[:], xnat[:, tb, P * kb : P * (kb + 1)], ident[:]
                    )
                    nc.vector.tensor_copy(
                        out=xT[:, kb, P * tb : P * (tb + 1)], in_=pt[:]
                    )

            # router logits (fp32 exact): logits[t, e] = x @ gate_w.T
            for tb in range(TB):
                pl = psA.tile([P, E], f32, tag="tp")
                for kb in range(KB):
                    nc.tensor.matmul(
                        pl[:],
                        xT[:, kb, P * tb : P * (tb + 1)],
                        gate_wT[:, kb, :],
                        start=(kb == 0),
                        stop=(kb == KB - 1),
                    )
                nc.vector.tensor_copy(out=logits[:, tb, :], in_=pl[:])

            # top-2 mask: mask = (l >= second_max(l))
            nc.vector.tensor_reduce(
                m1[:], logits[:], axis=mybir.AxisListType.X, op=mybir.AluOpType.max
            )
            nc.vector.tensor_tensor(
                eqbig[:],
                logits[:],
                m1[:, :, None].to_broadcast([P, TB, E]),
                mybir.AluOpType.is_equal,
            )
            nc.vector.tensor_scalar_mul(eqbig[:], eqbig[:], BIG)
            nc.vector.tensor_tensor(
                l2[:], logits[:], eqbig[:], mybir.AluOpType.subtract
            )
            nc.vector.tensor_reduce(
                m2[:], l2[:], axis=mybir.AxisListType.X, op=mybir.AluOpType.max
            )
            nc.vector.tensor_tensor(
                mask[:],
                logits[:],
                m2[:, :, None].to_broadcast([P, TB, E]),
                mybir.AluOpType.is_ge,
            )

            # mask^T: [256, 8] -> [8, 256], then DMA rows onto partition 0
            for tb in range(TB):
                pt = psA.tile([P, P], f32, tag="tp")
                nc.tensor.transpose(pt[:8, :], mask[:, tb, :], ident[:])
                nc.vector.tensor_copy(out=mtmp[:, tb, :], in_=pt[:8, :])
            for tb in range(TB):
                nc.sync.dma_start(
                    maskT[0:1, :, P * tb : P * (tb + 1)], mtmp[:, tb, :]
                )

            # mask broadcast tiles via PE outer product (ones^T x maskT[e])
            for e in range(E):
                pb = psA.tile([P, TL], f32, tag="tp")
                nc.tensor.matmul(
                    pb[:], ones1[:], maskT[0:1, e, :], start=True, stop=True
                )
                nc.vector.tensor_copy(out=mbc[:, e, :], in_=pb[:])

            psA_cm.__exit__(None, None, None)

            # ---- phase 1 matmuls: W_c = up^T.T @ down  (bf16) ----
            # nch0 uses all 8 PSUM banks with k emitted in DMA arrival order,
            # so W[:, :512] completes as the last weight quarter lands and
            # AG0 fires immediately; nch1 then runs at full PE rate.
            psW_cm = tc.tile_pool(name="psW", bufs=8, space="PSUM")
            psW = psW_cm.__enter__()
            for nch in range(NCH):
                pw = [
                    psW.tile([P, 512], f32, tag="w", name=f"pw_{nch}_{j}")
                    for j in range(8)
                ]
                for k in range(KI):
                    for m in range(8):
                        nc.tensor.matmul(
                            pw[m][:],
                            upT[:, k, P * m : P * (m + 1)],
                            downs[nch][:, k, :],
                            start=(k == 0),
                            stop=(k == KI - 1),
                        )
                wev = ph1.tile([P, 8, 512], bf16, tag="wev")
                for m in range(8):
                    nc.vector.tensor_copy(out=wev[:, m, :], in_=pw[m][:])
                nc.scalar.dma_start(
                    ag_in[nch][:].rearrange("p (m n) -> p m n", m=KB), wev[:]
                )
                nc.gpsimd.collective_compute(
                    "AllGather",
                    mybir.AluOpType.bypass,
                    replica_groups=[list(range(N_CORES))],
                    ins=[ag_in[nch].opt()],
                    outs=[ag_out[nch].opt()],
                )
            psW_cm.__exit__(None, None, None)

            # close phase-1 pools; open apply pools
            ph1_cm.__exit__(None, None, None)
            big_cm.__exit__(None, None, None)

            ap_cm = tc.tile_pool(name="apply", bufs=3)
            ap = ap_cm.__enter__()
            xmp_cm = tc.tile_pool(name="xm", bufs=1)
            xmp = xmp_cm.__enter__()
            psO_cm = tc.tile_pool(name="psO", bufs=4, space="PSUM")
            psO = psO_cm.__enter__()

            pout = [
                psO.tile([P, 512], f32, tag="o", name=f"pout_{j}") for j in range(4)
            ]
            # masked copies of x^T for all experts, computed on DVE while the
            # first AllGather is in flight
            xms = []
            for e in range(E):
                xm = xmp.tile([P, KB, TL], bf16, tag=f"xm{e}", name=f"xm_{e}")
                for kb in range(KB):
                    nc.vector.tensor_tensor(
                        xm[:, kb, :],
                        xT[:, kb, :],
                        mbc[:, e, :],
                        mybir.AluOpType.mult,
                    )
                xms.append(xm)

            junk = psO.tile([P, 256], f32, tag="junk")
            for nch in range(NCH):
                if nch == 1:
                    for jf in range(80):
                        nc.tensor.matmul(
                            junk[:],
                            xms[jf % 8][:, jf % 8, 0:P],
                            xms[(jf + 1) % 8][:, (jf + 3) % 8, :],
                            start=True,
                            stop=True,
                        )
                for e in range(E):
                    we = ap.tile(
                        [P, KB, 512], bf16, tag="wstr", name=f"we_{nch}_{e}", bufs=4
                    )
                    src_e = ag_out[nch][e].rearrange("p (m n) -> p m n", m=KB)
                    nc.scalar.dma_start(we[:, 0:4, :], src_e[:, 0:4, :])
                    nc.scalar.dma_start(we[:, 4:8, :], src_e[:, 4:8, :])
                    xm = xms[e]
                    for mt in range(TB):
                        for kb in range(KB):
                            nc.tensor.matmul(
                                pout[2 * mt + nch][:],
                                xm[:, kb, P * mt : P * (mt + 1)],
                                we[:, kb, :],
                                start=(e == 0 and kb == 0),
                                stop=(e == E - 1 and kb == KB - 1),
                            )

            outsb = ap.tile([P, TB, D], f32, tag="outsb", bufs=1)
            for mt in range(TB):
                for nch in range(NCH):
                    nc.vector.tensor_copy(
                        out=outsb[:, mt, 512 * nch : 512 * (nch + 1)],
                        in_=pout[2 * mt + nch][:],
                    )
            nc.sync.dma_start(
                out_ext.rearrange("(b p) d -> p b d", p=P), outsb[:]
            )

            psO_cm.__exit__(None, None, None)
            xmp_cm.__exit__(None, None, None)
            ap_cm.__exit__(None, None, None)

    nc.finalize()
    return nc


def _get_nc():
    if "nc" not in _cached:
        _cached["nc"] = _build()
    return _cached["nc"]


def _make_in_maps(inputs):
    hs = np.asarray(inputs["hidden_states"], dtype=np.float32)
    gate_w = np.ascontiguousarray(np.asarray(inputs["gate_w"], dtype=np.float32))
    w_up = np.asarray(inputs["w_up"], dtype=np.float32)
    w_down = np.asarray(inputs["w_down"], dtype=np.float32)
    x = np.ascontiguousarray(hs.reshape(-1, D))
    in_maps = []
    for c in range(N_CORES):
        in_maps.append(
            {
                "hidden_states": np.ascontiguousarray(x[TL * c : TL * (c + 1)]),
                "gate_w": gate_w,
                # host layout prep: expert-shard w_up and store it transposed
                "w_upT": np.ascontiguousarray(w_up[c].T),
                "w_down": np.ascontiguousarray(w_down[c]),
            }
        )
    return in_maps, hs.shape


def kernel(**inputs) -> np.ndarray:
    from concourse.bass_utils import run_bass_kernel_spmd

    in_maps, orig_shape = _make_in_maps(inputs)
    nc = _get_nc()
    last_err = None
    for _attempt in range(3):
        try:
            res = run_bass_kernel_spmd(nc, in_maps, core_ids=list(range(N_CORES)))
            break
        except Exception as err:  # transient NRT/device hiccup: retry
            last_err = err
            import time as _time

            _time.sleep(2.0)
    else:
        raise last_err
    out = np.concatenate([res.results[c]["out"] for c in range(N_CORES)], axis=0)
    return out.reshape(orig_shape).astype(np.float32)


def run_traced(**inputs):
    """Like kernel() but returns (out, BassKernelResults with trace)."""
    from concourse.bass_utils import run_bass_kernel_spmd

    in_maps, orig_shape = _make_in_maps(inputs)
    nc = _get_nc()
    res = run_bass_kernel_spmd(
        nc, in_maps, core_ids=list(range(N_CORES)), trace=True
    )
    out = np.concatenate([res.results[c]["out"] for c in range(N_CORES)], axis=0)
    return out.reshape(orig_shape).astype(np.float32), res



# revision 5
# speedup vs baseline: 1.0861x; 1.0861x over previous
"""MoE routing kernel for TRN2, 8 NeuronCores (expert-parallel).

Math: out[t] = sum_{e in top2(logits[t])} x[t] @ w_up[e] @ w_down[e]
(reference applies no activation between projections and no prob weighting,
so each expert collapses to one fused matrix W_e = w_up[e] @ w_down[e]).

Distribution / schedule (v2):
  Host prep: weights cast to bf16, x and gate transposed on host.
  Phase 1 (expert parallel): core c computes W_c = up_c @ down_c in four
    row-stages of 256 rows each (m-pair stages; PSUM 2 stages x 4 banks).
    upT streams per-stage (32KB resident), down stays resident (64KB).
    After each stage: DVE evac to bf16 -> AllGather of that 256-row slice.
    The four AGs pipeline behind the remaining W stages.
  Phase 2 (token parallel): core c owns tokens [256c, 256(c+1)); fp32
    router + top-2 mask computed up front (during weight DMA), masked
    x^T copies (xm_e, bf16) built on DVE. Apply accumulates all 8 experts
    x 8 d1-tiles into PSUM, consuming AG output slices streamed on the
    scalar DMA queue. No junk matmuls; PE should never idle after ~10us.
"""

import numpy as np

E = 8
TOPK = 2
D = 1024
I = 4096
T = 2048  # 4*512 tokens
N_CORES = 8
TL = T // N_CORES  # tokens per core (256)
P = 128
BIG = 1.0e30

NQ = 4            # W row-stages
MQ = 2            # m-tiles (128 rows) per stage
KB = D // P       # 8  d1 tiles
KI = I // P       # 32 contraction tiles
TB = TL // P      # 2  token tiles

_cached = {}


def _build():
    import concourse.bass as bass  # noqa: F401
    import concourse.tile as tile
    from concourse import bacc, mybir
    from concourse.masks import make_identity

    f32 = mybir.dt.float32
    bf16 = mybir.dt.bfloat16

    nc = bacc.Bacc("TRN2", target_bir_lowering=False, debug=False, num_devices=N_CORES)
    xT_ext = nc.declare_dram_parameter("xT", [D, TL], f32, isOutput=False)
    gateT_ext = nc.declare_dram_parameter("gateT", [D, E], f32, isOutput=False)
    upT_ext = nc.declare_dram_parameter("w_upT", [I, D], bf16, isOutput=False)
    down_ext = nc.declare_dram_parameter("w_down", [I, D], bf16, isOutput=False)
    out_ext = nc.declare_dram_parameter("out", [TL, D], f32, isOutput=True)

    with tile.TileContext(nc) as tc:
        import contextlib

        with contextlib.ExitStack() as ctx:
            const = ctx.enter_context(tc.tile_pool(name="const", bufs=1))
            dram = ctx.enter_context(tc.tile_pool(name="dram", bufs=1, space="DRAM"))
            xmp = ctx.enter_context(tc.tile_pool(name="xm", bufs=1))

            ident = const.tile([P, P], f32)
            make_identity(nc, ident[:])
            ones1 = const.tile([1, P], f32)
            nc.vector.memset(ones1[:], 1.0)

            # AG bounce buffers, one per W row-stage (256 rows x 1024 cols bf16).
            ag_in = []
            ag_out = []
            for q in range(NQ):
                gi = dram.tile([P, MQ * D], bf16, name=f"ag_in_{q}")
                go = dram.tile([E, P, MQ * D], bf16, addr_space="Shared", name=f"ag_out_{q}")
                ag_in.append(gi)
                ag_out.append(go)

            xms = []
            for e in range(E):
                xm = xmp.tile([P, KB, TL], bf16, tag=f"xm{e}", name=f"xm_{e}")
                xms.append(xm)

            # ---- router-scope pool (closes before the W phase needs SBUF) ----
            rt_cm = tc.tile_pool(name="router", bufs=1)
            rt = rt_cm.__enter__()
            psA_cm = tc.tile_pool(name="psA", bufs=4, space="PSUM")
            psA = psA_cm.__enter__()

            xT = rt.tile([P, KB, TL], f32)
            nc.sync.dma_start(xT[:], xT_ext.rearrange("(kb p) t -> p kb t", p=P))
            gateT = rt.tile([P, KB, E], f32)
            nc.sync.dma_start(gateT[:], gateT_ext.rearrange("(kb p) e -> p kb e", p=P))

            logits = rt.tile([P, TB, E], f32)
            m1 = rt.tile([P, TB], f32)
            eqbig = rt.tile([P, TB, E], f32)
            l2 = rt.tile([P, TB, E], f32)
            m2 = rt.tile([P, TB], f32)
            mask = rt.tile([P, TB, E], f32)
            mtmp = rt.tile([8, TB, P], f32)
            maskT = rt.tile([1, E, TL], f32)
            mbc = rt.tile([P, E, TL], f32)

            # router logits (fp32 exact): logits[t, e] = x @ gate_w.T
            for tb in range(TB):
                pl = psA.tile([P, E], f32, tag="tp")
                for kb in range(KB):
                    nc.tensor.matmul(
                        pl[:],
                        xT[:, kb, P * tb : P * (tb + 1)],
                        gateT[:, kb, :],
                        start=(kb == 0),
                        stop=(kb == KB - 1),
                    )
                nc.vector.tensor_copy(out=logits[:, tb, :], in_=pl[:])

            # top-2 mask: mask = (l >= second_max(l))
            nc.vector.tensor_reduce(
                m1[:], logits[:], axis=mybir.AxisListType.X, op=mybir.AluOpType.max
            )
            nc.vector.tensor_tensor(
                eqbig[:],
                logits[:],
                m1[:, :, None].to_broadcast([P, TB, E]),
                mybir.AluOpType.is_equal,
            )
            nc.vector.tensor_scalar_mul(eqbig[:], eqbig[:], BIG)
            nc.vector.tensor_tensor(l2[:], logits[:], eqbig[:], mybir.AluOpType.subtract)
            nc.vector.tensor_reduce(
                m2[:], l2[:], axis=mybir.AxisListType.X, op=mybir.AluOpType.max
            )
            nc.vector.tensor_tensor(
                mask[:],
                logits[:],
                m2[:, :, None].to_broadcast([P, TB, E]),
                mybir.AluOpType.is_ge,
            )

            # mask^T: [256, 8] -> [8, 256] on partition 0, then bcast via PE
            for tb in range(TB):
                pt = psA.tile([P, P], f32, tag="tp")
                nc.tensor.transpose(pt[:8, :], mask[:, tb, :], ident[:])
                nc.vector.tensor_copy(out=mtmp[:, tb, :], in_=pt[:8, :])
            for tb in range(TB):
                nc.sync.dma_start(maskT[0:1, :, P * tb : P * (tb + 1)], mtmp[:, tb, :])
            for e in range(E):
                pb = psA.tile([P, TL], f32, tag="tp")
                nc.tensor.matmul(pb[:], ones1[:], maskT[0:1, e, :], start=True, stop=True)
                nc.vector.tensor_copy(out=mbc[:, e, :], in_=pb[:])

            # masked x^T per expert (bf16), all on DVE during the weight DMA
            for e in range(E):
                for kb in range(KB):
                    nc.vector.tensor_tensor(
                        xms[e][:, kb, :],
                        xT[:, kb, :],
                        mbc[:, e, :],
                        mybir.AluOpType.mult,
                    )

            psA_cm.__exit__(None, None, None)
            rt_cm.__exit__(None, None, None)

            # ---- weight pools ----
            wevp = ctx.enter_context(tc.tile_pool(name="wev", bufs=2))
            wep = ctx.enter_context(tc.tile_pool(name="wep", bufs=8))

            dn_cm = tc.tile_pool(name="dn", bufs=1)
            dn = dn_cm.__enter__()
            down = dn.tile([P, KI, D], bf16)
            # k-major group loads on the scalar queue (paces W stage 0)
            for g in range(8):
                nc.scalar.dma_start(
                    down[:, 4 * g : 4 * (g + 1), :],
                    down_ext[512 * g : 512 * (g + 1), :].rearrange(
                        "(ko p) n -> p ko n", p=P
                    ),
                )

            up_cm = tc.tile_pool(name="up", bufs=4)
            up = up_cm.__enter__()
            upq = []  # [q][half] -> [P, 16, 256] bf16
            for q in range(NQ):
                halves = []
                for h in range(2):
                    t = up.tile([P, KI // 2, P * MQ], bf16, tag="upq", name=f"upq_{q}_{h}")
                    nc.sync.dma_start(
                        t[:],
                        upT_ext[
                            2048 * h : 2048 * (h + 1), 256 * q : 256 * (q + 1)
                        ].rearrange("(ko p) n -> p ko n", p=P),
                    )
                    halves.append(t)
                upq.append(halves)

            wes = [[None] * E for _ in range(NQ)]

            # ---- phase 1: W_c row-stages + pipelined AllGathers ----
            psW_cm = tc.tile_pool(name="psW", bufs=2, space="PSUM")
            psW = psW_cm.__enter__()
            for q in range(NQ):
                pw = [
                    [psW.tile([P, 512], f32, tag=f"pw{m2}{ch}", name=f"pw_{q}_{m2}_{ch}") for ch in range(2)]
                    for m2 in range(MQ)
                ]
                for k in range(KI):
                    lhs = upq[q][k // 16]
                    for m2 in range(MQ):
                        for ch in range(2):
                            nc.tensor.matmul(
                                pw[m2][ch][:],
                                lhs[:, k % 16, P * m2 : P * (m2 + 1)],
                                down[:, k, 512 * ch : 512 * (ch + 1)],
                                start=(k == 0),
                                stop=(k == KI - 1),
                            )
                wev = wevp.tile([P, MQ, D], bf16, tag="wev", name=f"wev_{q}")
                for m2 in range(MQ):
                    for ch in range(2):
                        nc.vector.tensor_copy(
                            out=wev[:, m2, 512 * ch : 512 * (ch + 1)],
                            in_=pw[m2][ch][:],
                        )
                nc.sync.dma_start(ag_in[q][:].rearrange("p (m n) -> p m n", m=MQ), wev[:])
                nc.gpsimd.collective_compute(
                    "AllGather",
                    mybir.AluOpType.bypass,
                    replica_groups=[list(range(N_CORES))],
                    ins=[ag_in[q].opt()],
                    outs=[ag_out[q].opt()],
                )
                # stream this stage's gathered slices in on the scalar queue
                for e in range(E):
                    we = wep.tile([P, MQ, D], bf16, tag="we", name=f"we_{q}_{e}")
                    nc.scalar.dma_start(
                        we[:], ag_out[q][e].rearrange("p (m n) -> p m n", m=MQ)
                    )
                    wes[q][e] = we
            psW_cm.__exit__(None, None, None)
            up_cm.__exit__(None, None, None)
            dn_cm.__exit__(None, None, None)

            # ---- phase 2: apply ----
            ap_cm = tc.tile_pool(name="apply", bufs=1)
            ap = ap_cm.__enter__()
            psO_cm = tc.tile_pool(name="psO", bufs=1, space="PSUM")
            psO = psO_cm.__enter__()

            pout = [
                [psO.tile([P, 512], f32, tag=f"o{tt}{ch}", name=f"pout_{tt}_{ch}") for ch in range(2)]
                for tt in range(TB)
            ]
            for q in range(NQ):
                for e in range(E):
                    we = wes[q][e]
                    for kbq in range(MQ):
                        for tt in range(TB):
                            for ch in range(2):
                                nc.tensor.matmul(
                                    pout[tt][ch][:],
                                    xms[e][:, MQ * q + kbq, P * tt : P * (tt + 1)],
                                    we[:, kbq, 512 * ch : 512 * (ch + 1)],
                                    start=(q == 0 and e == 0 and kbq == 0),
                                    stop=(q == NQ - 1 and e == E - 1 and kbq == MQ - 1),
                                )

            outsb = ap.tile([P, TB, D], f32, tag="outsb")
            for tt in range(TB):
                for ch in range(2):
                    nc.vector.tensor_copy(
                        out=outsb[:, tt, 512 * ch : 512 * (ch + 1)],
                        in_=pout[tt][ch][:],
                    )
            nc.sync.dma_start(out_ext.rearrange("(b p) d -> p b d", p=P), outsb[:])

            psO_cm.__exit__(None, None, None)
            ap_cm.__exit__(None, None, None)

    nc.finalize()
    return nc


def _get_nc():
    if "nc" not in _cached:
        _cached["nc"] = _build()
    return _cached["nc"]


def _make_in_maps(inputs):
    import ml_dtypes

    bf16 = ml_dtypes.bfloat16
    hs = np.asarray(inputs["hidden_states"], dtype=np.float32)
    gate_w = np.asarray(inputs["gate_w"], dtype=np.float32)
    w_up = np.asarray(inputs["w_up"], dtype=np.float32)
    w_down = np.asarray(inputs["w_down"], dtype=np.float32)
    x = hs.reshape(-1, D)
    gateT = np.ascontiguousarray(gate_w.T)
    in_maps = []
    for c in range(N_CORES):
        in_maps.append(
            {
                "xT": np.ascontiguousarray(x[TL * c : TL * (c + 1)].T),
                "gateT": gateT,
                "w_upT": np.ascontiguousarray(w_up[c].T).astype(bf16),
                "w_down": np.ascontiguousarray(w_down[c]).astype(bf16),
            }
        )
    return in_maps, hs.shape


def kernel(**inputs) -> np.ndarray:
    from concourse.bass_utils import run_bass_kernel_spmd

    in_maps, orig_shape = _make_in_maps(inputs)
    nc = _get_nc()
    last_err = None
    for _attempt in range(3):
        try:
            res = run_bass_kernel_spmd(nc, in_maps, core_ids=list(range(N_CORES)))
            break
        except Exception as err:  # transient NRT/device hiccup: retry
            last_err = err
            import time as _time

            _time.sleep(2.0)
    else:
        raise last_err
    out = np.concatenate([res.results[c]["out"] for c in range(N_CORES)], axis=0)
    return out.reshape(orig_shape).astype(np.float32)


def run_traced(**inputs):
    """Like kernel() but returns (out, BassKernelResults with trace)."""
    from concourse.bass_utils import run_bass_kernel_spmd

    in_maps, orig_shape = _make_in_maps(inputs)
    nc = _get_nc()
    res = run_bass_kernel_spmd(nc, in_maps, core_ids=list(range(N_CORES)), trace=True)
    out = np.concatenate([res.results[c]["out"] for c in range(N_CORES)], axis=0)
    return out.reshape(orig_shape).astype(np.float32), res
